# revision 17
# baseline (speedup 1.0000x reference)
"""CompactBilinearPooling kernel for Trainium2 (8 NeuronCores, SPMD data-parallel).

Per core (32 batch rows):
  1. Count-sketch both inputs into a parity-split fp16 SBUF table pair
     (A = even bin%128, B = odd) via dma_scatter_add's SBUF-dst CCE-add
     mode with idx' = (bin%128)*128 + bin//128, so partition = bin//128
     and the FFT reads the table in place. The host sorts x columns by
     collision rank (padded to 128-col chunks) and orders each rank
     class so class r+1's token at window position j has its same-bin
     class-r partner at position j; partition-aligned DVE chunk adds
     then fold every rank>=1 payload down into the rank-0 columns (pad
     columns are zero), leaving ONE scatter instruction whose tokens all
     hit distinct bins (~8.5us fixed Q7 dispatch + ~7ns/token desc-gen).
  2. Circular convolution via FFT packing: Z = FFT(y1 + i*y2),
     out = Im(IFFT(Z^2))/2. Length-16384 FFT = 128x128 four-step, fp16
     in / f32 PSUM matmuls, with 1/sqrt(2*16384) folded into the first
     DFT so all intermediates fit fp16. Stage outputs are evacuated by
     ACT into flat fp16 tiles (re/im h-major in PSUM via strided matmul
     outs), and ALL twiddle/recombine/square elementwise ops run on DVE
     as dense 16-bit tensor_tensor (2x uop), fused across group PAIRS
     ([128,1024] ops) to amortize the per-op overhead. GPSIMD does no
     FFT work: concurrent DVE+GPSIMD SBUF traffic was measured to slow
     both ~2.3x. The parity split is a fixed permutation pi of the
     inner FFT digit, absorbed host-side into twiddle/weight row order.
"""
import sys

sys.path.insert(0, "/opt/trn_rl_repo")

import numpy as np

import concourse.bass as bass
import concourse.bacc as bacc
import concourse.mybir as mybir
import concourse.tile as tile
from concourse.bass_utils import run_bass_kernel_spmd

P = 128
B, D, O = 256, 4096, 16384
NCORES = 8
BC = B // NCORES          # 32 rows per core
BC2 = 2 * BC              # x1|x2 stacked on partitions
F32R = mybir.dt.float32r
F32 = mybir.dt.float32
F16 = mybir.dt.float16

# packed fp16 consts (t-tables pair-tiled to 8*P)
_CON = {}
_off = 0
for _name, _w in [("wa1", 2 * P), ("wa2", 2 * P), ("wfre", P), ("wfim", P),
                  ("wfimn", P), ("wi1", 2 * P), ("wi2", 2 * P), ("wire", P),
                  ("wiim", P), ("t1re", P), ("t1im", P),
                  ("t2re", P), ("t2im", P)]:
    _CON[_name] = (_off, _off + _w)
    _off += _w
NCON = _off

_cache = {}


def _build(rounds, folds, nt):
    """rounds: ((n_chunks, count), ...) scatter rounds (usually one).
    folds: ((src_chunk, dst_chunk), ...) payload folds, applied in order.
    nt: total chunks in the padded layout."""
    icols = sum(nch * 8 for nch, _ in rounds)

    nc = bacc.Bacc("TRN2", target_bir_lowering=False, debug=False)

    x12 = nc.dram_tensor("x12", [BC2, nt * P], F16, kind="ExternalInput")
    identm = nc.dram_tensor("identm", [BC2, BC2], F16, kind="ExternalInput")
    idxs = nc.dram_tensor("idxs", [P, icols], mybir.dt.int16, kind="ExternalInput")
    sTb = nc.dram_tensor("sTb", [P, nt * 64], F16, kind="ExternalInput")
    fftc = nc.dram_tensor("fftc", [P, NCON], F16, kind="ExternalInput")
    out = nc.dram_tensor("out", [BC, O], F32, kind="ExternalOutput")

    with tile.TileContext(nc) as tc:
        with (
            tc.tile_pool(name="const", bufs=1) as cp,
            tc.tile_pool(name="work", bufs=1) as wp,
            tc.tile_pool(name="tmp", bufs=2) as tp,
            tc.tile_pool(name="psum", bufs=4, space="PSUM") as pp,
        ):
            # ---- loads, priority order on one FIFO queue ----
            xs12 = wp.tile([BC2, nt * P], F16, tag="xs12")
            nc.sync.dma_start(xs12[:], x12[:])
            ident_t = cp.tile([BC2, BC2], F16, tag="ident")
            nc.sync.dma_start(ident_t[:], identm[:])
            idxs_s = cp.tile([P, icols], mybir.dt.int16, tag="idxs")
            nc.sync.dma_start(idxs_s[:], idxs[:])
            sTb_s = cp.tile([P, nt * 64], F16, tag="sTb")
            nc.sync.dma_start(sTb_s[:], sTb[:])
            fftc_s = cp.tile([P, NCON], F16, tag="fftc")
            nc.sync.dma_start(fftc_s[:], fftc[:])

            def con(name):
                a, b = _CON[name]
                return fftc_s[:, a:b]

            wa1_s, wa2_s = con("wa1"), con("wa2")
            wfre_s, wfim_s, wfimn_s = con("wfre"), con("wfim"), con("wfimn")
            wi1_s, wi2_s = con("wi1"), con("wi2")
            wire_s, wiim_s = con("wire"), con("wiim")
            ident = ident_t[:]

            # twiddle tables arrive as raw [P, P]; replicate x8 on DVE via
            # log-doubling (runs during scatter desc-gen, off the DMA front)
            t8 = cp.tile([P, 4 * 1024], F16, tag="t8")
            for ti, name in enumerate(("t1re", "t1im", "t2re", "t2im")):
                base = ti * 1024
                nc.vector.tensor_copy(t8[:, base:base + P], con(name))
                for w in (P, 2 * P, 4 * P):
                    nc.vector.tensor_copy(t8[:, base + w:base + 2 * w],
                                          t8[:, base:base + w])
            t1re_s, t1im_s = t8[:, 0:1024], t8[:, 1024:2048]
            t2re_s, t2im_s = t8[:, 2048:3072], t8[:, 3072:4096]

            # ---- parity-split fp16 sketch table in SBUF ----
            y2 = wp.tile([P, 2 * 64 * 32], F32, tag="y2")   # fp16 pair view
            tabA = y2[:, 0:2048].bitcast(F16)
            tabB = y2[:, 2048:4096].bitcast(F16)

            nc.vector.memset(y2[:, 0:2048], 0.0)
            nc.gpsimd.memset(y2[:, 2048:4096], 0.0)

            # ---- PE transposes -> sxT [d%128, (chunk, [y1|y2] rows)] ----
            sxT = wp.tile([P, nt * 64], F16, tag="sxT")
            groups = [(g * 16, min(16, nt - g * 16)) for g in range((nt + 15) // 16)]
            for g0, gn in groups:
                ps = pp.tile([P, 1024], F16, space="PSUM", tag="psd", bufs=2)
                for jj in range(gn):
                    j = g0 + jj
                    nc.tensor.transpose(out=ps[:, jj * 64:(jj + 1) * 64],
                                        in_=xs12[:, j * P:(j + 1) * P],
                                        identity=ident)
                nc.vector.tensor_mul(sxT[:, g0 * 64:(g0 + gn) * 64],
                                     ps[:, :gn * 64],
                                     sTb_s[:, g0 * 64:(g0 + gn) * 64])

            # ---- fold rank>=1 payloads down into rank-0 partner columns ----
            for src_c, dst_c in folds:
                nc.vector.tensor_add(sxT[:, dst_c * 64:(dst_c + 1) * 64],
                                     sxT[:, dst_c * 64:(dst_c + 1) * 64],
                                     sxT[:, src_c * 64:(src_c + 1) * 64])

            # ---- scatter (SBUF-dst parity-split CCE add, fp16) ----
            cs, ioff = 0, 0
            for nch, cnt in rounds:
                win = nch * P
                nc.gpsimd.dma_scatter_add(
                    out_ap=tabA,
                    out_ap_other=tabB,
                    in_ap=sxT[:, cs * 64:(cs + nch) * 64].rearrange(
                        "p (t e) -> p t e", e=64),
                    idxs_ap=idxs_s[:, ioff:ioff + win // 16],
                    num_idxs=win,
                    num_idxs_reg=cnt,
                    elem_size=64,
                    sbuf_tokens_per_rank=P,
                    parity_reg=0,
                )
                cs += nch
                ioff += win // 16

            # ---- FFT reads the table in place: lhsT [q, (par, g)] where
            # (par, g) linearizes to pi(n) = (n%2)*64 + n//2 ----
            y2r = y2[:].bitcast(F16).rearrange("q (par g e) -> q par g e",
                                               par=2, e=64)

            ssb_re = wp.tile([P, P * BC], F16, tag="ssb_re")
            ssb_im = wp.tile([P, P * BC], F16, tag="ssb_im")
            osb = wp.tile([P, P * BC], F32, tag="osb")
            mts, nts = {}, {}

            def halves(ps):
                v = ps[:].rearrange("p (b2 h k) -> p b2 h k", b2=4, h=2)
                return v[:, :, 0, :], v[:, :, 1, :]

            def pair_a(pg):
                presP = tp.tile([P, 1024], F16, tag="presP")
                pimsP = tp.tile([P, 1024], F16, tag="pimsP")
                for gh in range(2):
                    g = 2 * pg + gh
                    ps = pp.tile([P, 1024], F32, space="PSUM", tag="ps", bufs=3)
                    for bb in range(4):
                        b_ = g * 4 + bb
                        sl = ps[:, bb * 256:(bb + 1) * 256]
                        nc.tensor.matmul(out=sl, lhsT=y2r[:, :, :, b_],
                                         rhs=wa1_s, start=True, stop=False)
                        nc.tensor.matmul(out=sl, lhsT=y2r[:, :, :, 32 + b_],
                                         rhs=wa2_s, start=False, stop=True)
                    pre, pim = halves(ps)
                    hs = slice(gh * 512, (gh + 1) * 512)
                    r3h = lambda t: t[:, hs].rearrange("p (b2 k) -> p b2 k", b2=4)
                    nc.scalar.copy(r3h(presP), pre)
                    nc.scalar.copy(r3h(pimsP), pim)
                a1 = tp.tile([P, 1024], F16, tag="m1")
                a2 = tp.tile([P, 1024], F16, tag="m2")
                a3 = tp.tile([P, 1024], F16, tag="m3")
                a4 = tp.tile([P, 1024], F16, tag="m4")
                mreP = tp.tile([P, 1024], F16, tag="mreP")
                mim2P = tp.tile([P, 1024], F16, tag="mim2P")
                nc.vector.tensor_mul(a1[:], presP[:], t1re_s)
                nc.vector.tensor_mul(a2[:], pimsP[:], t1im_s)
                nc.vector.tensor_sub(mreP[:], a1[:], a2[:])
                nc.vector.tensor_mul(a3[:], presP[:], t1im_s)
                nc.vector.tensor_mul(a4[:], pimsP[:], t1re_s)
                nc.vector.tensor_add(mim2P[:], a3[:], a4[:])
                mts[pg] = (mreP, mim2P)

            def pair_b(pg):
                mreP, mim2P = mts.pop(pg)
                zresP = tp.tile([P, 1024], F16, tag="zresP")
                zimsP = tp.tile([P, 1024], F16, tag="zimsP")
                for gh in range(2):
                    hs = slice(gh * 512, (gh + 1) * 512)
                    ps = pp.tile([P, 1024], F32, space="PSUM", tag="ps", bufs=3)
                    zre, zim = ps[:, 0:512], ps[:, 512:1024]
                    nc.tensor.matmul(out=zre, lhsT=wfre_s, rhs=mreP[:, hs],
                                     start=True, stop=False)
                    nc.tensor.matmul(out=zre, lhsT=wfimn_s, rhs=mim2P[:, hs],
                                     start=False, stop=True)
                    nc.tensor.matmul(out=zim, lhsT=wfim_s, rhs=mreP[:, hs],
                                     start=True, stop=False)
                    nc.tensor.matmul(out=zim, lhsT=wfre_s, rhs=mim2P[:, hs],
                                     start=False, stop=True)
                    nc.scalar.copy(zresP[:, hs], zre)
                    nc.scalar.copy(zimsP[:, hs], zim)
                rs = slice(pg * 1024, (pg + 1) * 1024)
                sp = tp.tile([P, 1024], F16, tag="m1")
                sm = tp.tile([P, 1024], F16, tag="m2")
                nc.vector.tensor_add(sp[:], zresP[:], zimsP[:])
                nc.vector.tensor_sub(sm[:], zresP[:], zimsP[:])
                nc.vector.tensor_mul(ssb_re[:, rs], sp[:], sm[:])
                nc.vector.tensor_mul(ssb_im[:, rs], zresP[:], zimsP[:])

            def pair_c(pg):
                preCsP = tp.tile([P, 1024], F16, tag="preCsP")
                pimCsP = tp.tile([P, 1024], F16, tag="pimCsP")
                for gh in range(2):
                    g = 2 * pg + gh
                    ps = pp.tile([P, 1024], F32, space="PSUM", tag="ps", bufs=3)
                    for bb in range(4):
                        b_ = g * 4 + bb
                        sl = ps[:, bb * 256:(bb + 1) * 256]
                        lre = ssb_re[:, b_ * P:(b_ + 1) * P]
                        lim = ssb_im[:, b_ * P:(b_ + 1) * P]
                        nc.tensor.matmul(out=sl, lhsT=lre, rhs=wi1_s,
                                         start=True, stop=False)
                        nc.tensor.matmul(out=sl, lhsT=lim, rhs=wi2_s,
                                         start=False, stop=True)
                    preC, pimC = halves(ps)
                    hs = slice(gh * 512, (gh + 1) * 512)
                    r3h = lambda t: t[:, hs].rearrange("p (b2 k) -> p b2 k", b2=4)
                    nc.scalar.copy(r3h(preCsP), preC)
                    nc.scalar.copy(r3h(pimCsP), pimC)
                c1 = tp.tile([P, 1024], F16, tag="n1")
                c2 = tp.tile([P, 1024], F16, tag="n2")
                c3 = tp.tile([P, 1024], F16, tag="n3")
                c4 = tp.tile([P, 1024], F16, tag="n4")
                nreP = tp.tile([P, 1024], F16, tag="nreP")
                nim2P = tp.tile([P, 1024], F16, tag="nim2P")
                nc.vector.tensor_mul(c1[:], preCsP[:], t2re_s)
                nc.vector.tensor_mul(c2[:], pimCsP[:], t2im_s)
                nc.vector.tensor_sub(nreP[:], c1[:], c2[:])
                nc.vector.tensor_mul(c3[:], preCsP[:], t2im_s)
                nc.vector.tensor_mul(c4[:], pimCsP[:], t2re_s)
                nc.vector.tensor_add(nim2P[:], c3[:], c4[:])
                nts[pg] = (nreP, nim2P)

            def pair_d(pg):
                nreP, nim2P = nts.pop(pg)
                for gh in range(2):
                    g = 2 * pg + gh
                    hs = slice(gh * 512, (gh + 1) * 512)
                    rs = slice(g * 512, (g + 1) * 512)
                    ps = pp.tile([P, 512], F32, space="PSUM", tag="psd", bufs=2)
                    po = ps[:, 0:512]
                    nc.tensor.matmul(out=po, lhsT=wiim_s, rhs=nreP[:, hs],
                                     start=True, stop=False)
                    nc.tensor.matmul(out=po, lhsT=wire_s, rhs=nim2P[:, hs],
                                     start=False, stop=True)
                    nc.scalar.copy(osb[:, rs], po)
                    nc.sync.dma_start(
                        out[:].rearrange("b (a c) -> a b c", c=P)[:, g * 4:(g + 1) * 4, :],
                        osb[:, rs].rearrange("a (b c) -> a b c", c=P))

            for t in range(7):
                if t < 4:
                    pair_a(t)
                if 1 <= t < 5:
                    pair_b(t - 1)
                if 2 <= t < 6:
                    pair_c(t - 2)
                if 3 <= t:
                    pair_d(t - 3)

    nc.compile()
    return nc


# pi(n) = (n%2)*64 + n//2 is the table's inner-digit order; row p of a
# permuted matrix holds the row for n = inv_pi(p) = 2*(p%64) + p//64
_PI_INV = np.array([2 * (p % 64) + p // 64 for p in range(P)])


def _host_consts():
    j = np.arange(P)
    ang = -2.0 * np.pi * np.outer(j, j) / P
    wf_re, wf_im = np.cos(ang), np.sin(ang)
    wi_re, wi_im = np.cos(-ang), np.sin(-ang)
    tang = -2.0 * np.pi * np.outer(j, j) / O
    alpha = 1.0 / np.sqrt(2.0 * O)   # replaces the 1/(2N) ifft normalization

    parts = {
        "wa1": np.concatenate([wf_re, wf_im], axis=1) * alpha,
        "wa2": np.concatenate([-wf_im, wf_re], axis=1) * alpha,
        "wfre": wf_re[_PI_INV], "wfim": wf_im[_PI_INV], "wfimn": -wf_im[_PI_INV],
        "wi1": np.concatenate([wi_re, wi_im], axis=1),
        "wi2": np.concatenate([-2.0 * wi_im, 2.0 * wi_re], axis=1),
        "wire": wi_re, "wiim": wi_im,
        "t1re": np.cos(tang)[_PI_INV], "t1im": np.sin(tang)[_PI_INV],
        "t2re": np.cos(tang), "t2im": -np.sin(tang),
    }
    fftc = np.zeros((P, NCON), np.float16)
    for name, (a, b) in _CON.items():
        fftc[:, a:b] = parts[name].astype(np.float16)
    return dict(fftc=fftc, identm=np.eye(BC2, dtype=np.float16))


def _host_prep(h1, s1):
    """Rank-sorted, chunk-padded column layout; every rank>=1 class is
    folded (device-side chunk adds) into its rank-(r-1) partner columns,
    leaving a single all-distinct-bins scatter."""
    h1 = np.asarray(h1, dtype=np.int64)
    s1 = np.asarray(s1, dtype=np.float32)
    rank = np.zeros(D, np.int64)
    seen = {}
    for d in range(D):
        b = int(h1[d])
        rank[d] = seen.get(b, 0)
        seen[b] = int(rank[d]) + 1
    nr = int(rank.max()) + 1
    order = np.argsort(rank, kind="stable")
    starts = np.concatenate([[0], np.cumsum([int((rank == r).sum())
                                             for r in range(nr)])])
    classes = [order[starts[r]:starts[r + 1]] for r in range(nr)]
    # order class r so that the partner of class r+1's token at window
    # position j sits at class r's position j (pads have no constraint)
    for r in range(nr - 2, -1, -1):
        nxt = classes[r + 1]
        if len(nxt) == 0:
            continue
        pos_in_next = {int(h1[d]): i for i, d in enumerate(nxt)}
        keyed = sorted(range(len(classes[r])),
                       key=lambda i: pos_in_next.get(int(h1[classes[r][i]]),
                                                     1 << 30))
        classes[r] = classes[r][keyed]
    nchs = [(len(c) + P - 1) // P for c in classes]
    foldable = all(len(classes[r]) <= len(classes[r - 1]) and
                   nchs[r] <= nchs[r - 1] for r in range(1, nr))

    rounds, flat, src = [], [], []
    start_chunk = {}
    chunk = 0
    for r in range(nr):
        cls = classes[r]
        cnt = len(cls)
        if cnt == 0:
            continue
        nch = nchs[r]
        win = nch * P
        start_chunk[r] = chunk
        src.extend(cls.tolist())
        src.extend([-1] * (win - cnt))
        if r == 0 or not foldable:
            f = np.full(win, -1, np.int64)
            f[:cnt] = (h1[cls] % P) * P + h1[cls] // P   # sigma(bin)
            flat.append(f)
            rounds.append((nch, cnt))
        chunk += nch
    # fold deepest class first; class r adds chunk-wise into class r-1's
    # first chunks (partner positions are partition/chunk aligned)
    fold_pairs = []
    if foldable:
        for r in range(nr - 1, 0, -1):
            if r not in start_chunk:
                continue
            for i in range(nchs[r]):
                fold_pairs.append((start_chunk[r] + i, start_chunk[r - 1] + i))
    src = np.asarray(src, np.int64)
    dp_cols = src.shape[0]
    nt = dp_cols // P

    idxs = np.concatenate(
        [np.tile(f.astype(np.int16).reshape(-1, 16).T, (8, 1)) for f in flat],
        axis=1)

    s_pad = np.zeros(dp_cols, np.float32)
    valid = src >= 0
    s_pad[valid] = s1[src[valid]]
    # sTb[p, c*64+e] = s_pad[c*128+p]
    sTb = np.ascontiguousarray(
        np.broadcast_to(s_pad.reshape(nt, P).T[:, :, None], (P, nt, 64))
    ).reshape(P, nt * 64).astype(np.float16)
    return tuple(rounds), tuple(fold_pairs), nt, src, idxs, sTb


_last_results = None


def kernel(x1, x2, h1, s1, output_size=O, **kw):
    global _last_results
    x1 = np.asarray(x1, np.float32)
    x2 = np.asarray(x2, np.float32)
    rounds, folds, nt, src, idxs, sTb = _host_prep(h1, s1)
    key = (rounds, folds, nt)
    if key not in _cache:
        _cache[key] = _build(rounds, folds, nt)
    nc = _cache[key]
    consts = _host_consts()
    valid = src >= 0
    in_maps = []
    for c in range(NCORES):
        m = dict(consts)
        x12 = np.zeros((BC2, nt * P), np.float16)
        x12[:BC, valid] = x1[c * BC:(c + 1) * BC][:, src[valid]]
        x12[BC:, valid] = x2[c * BC:(c + 1) * BC][:, src[valid]]
        m["x12"] = x12
        m["idxs"] = idxs
        m["sTb"] = sTb
        in_maps.append(m)
    res = run_bass_kernel_spmd(nc, in_maps, core_ids=list(range(NCORES)))
    _last_results = res
    return np.concatenate([res.results[c]["out"] for c in range(NCORES)], axis=0)
